# revision 38
# baseline (speedup 1.0000x reference)
"""Trainium2 Bass kernel for nn_AffineCouplingLayer.

Reference computation (B=65536, D=256, C=128, H=1024, SPLIT=128):
    x1u = x[:, 0::2]; x2u = x[:, 1::2]
    h = relu([x1u, cond] @ W1 + b1)
    h = relu(h @ W2 + b2)
    log_s = tanh(h @ Ws + bs);  t = h @ Wt + bt
    x2t = x2u * exp(log_s) + t
    out[:, 0::2] = x1u; out[:, 1::2] = x2t
    log_det = log_s.sum(axis=1)

Strategy: pure data parallel over 8 cores (batch dim), weights replicated.
On-chip layout is feature-major ("transposed"): activations live as
[feature(partition=128), batch(free)] tiles so every layer's output is
directly the next layer's matmul rhs with zero transposes. Matmuls run in
bf16 (full PE rate vs 1/4 for fp32) with fp32 PSUM accumulation. The host
pre-transposes the inputs / un-transposes the outputs and passes
x[:, 0::2] through untouched (only the odd columns are computed on device).
log_det (a partition-dim reduction) is a ones-vector bf16 matmul.

PSUM evacuations alternate between ScalarE (ACT, fused bias+relu) and
VectorE (DVE scalar_tensor_tensor, (psum+bias) max 0) so consecutive
chunks drain in parallel and neither engine gates the PE. Input loads run
on the GpSimd queue, weight loads on Sync, so the startup DMA wall
parallelizes; weight tensors are loaded with one coalesced DMA each.
"""

import numpy as np
import ml_dtypes

N_CORES = 8
B, D, C, H = 65536, 256, 128, 1024
SPLIT = D // 2            # 128
NET_IN = (D - SPLIT) + C  # 256
BS = B // N_CORES         # 8192 rows per core
NT = 512                  # batch-tile (matmul moving free dim, 1 PSUM bank fp32)
NTILES = BS // NT         # 16
KH = H // 128             # 8 contraction chunks for H
MH = H // 128             # 8 output chunks for H
KIN = NET_IN // 128       # 2 contraction chunks for layer 1

_CACHE = {}


def _build_nc():
    import concourse.mybir as mybir
    import concourse.tile as tile
    from concourse import bacc

    f32 = mybir.dt.float32
    bf16 = mybir.dt.bfloat16
    ACT = mybir.ActivationFunctionType
    ALU = mybir.AluOpType

    nc = bacc.Bacc("TRN2", target_bir_lowering=False, debug=False,
                   num_devices=N_CORES)

    # Per-core inputs (already transposed/cast host-side)
    a_inT = nc.dram_tensor("a_inT", [NET_IN, BS], bf16, kind="ExternalInput")
    x2uT = nc.dram_tensor("x2uT", [SPLIT, BS], f32, kind="ExternalInput")
    w1 = nc.dram_tensor("w1", [NET_IN, H], bf16, kind="ExternalInput")
    w2 = nc.dram_tensor("w2", [H, H], bf16, kind="ExternalInput")
    ws = nc.dram_tensor("ws", [H, SPLIT], bf16, kind="ExternalInput")
    wt = nc.dram_tensor("wt", [H, SPLIT], bf16, kind="ExternalInput")
    b12 = nc.dram_tensor("b12", [128, 2 * MH], f32, kind="ExternalInput")
    bst = nc.dram_tensor("bst", [SPLIT, 2], f32, kind="ExternalInput")

    x2tT = nc.dram_tensor("x2tT", [SPLIT, BS], f32, kind="ExternalOutput")
    # log_s ships raw (bf16); the host does the 128-feature sum. This keeps
    # the partition-dim reduction off the TensorE stream entirely (16 fewer
    # full-cost matmuls).
    log_sT = nc.dram_tensor("log_sT", [SPLIT, BS], bf16, kind="ExternalOutput")

    with tile.TileContext(nc) as tc:
        with (
            tc.tile_pool(name="wpool", bufs=1) as wpool,
            tc.tile_pool(name="io", bufs=3) as io,
            tc.tile_pool(name="hpool", bufs=2) as hpool,
            tc.tile_pool(name="misc", bufs=2) as misc,
            tc.tile_pool(name="psum", bufs=4, space="PSUM") as psum,
        ):
            # ---- resident weights / biases (coalesced DMAs on Sync) ----
            # The first real matmul needs tile-0's activations + w1's head;
            # split tile-0's input across the fast Sync queue (k=0 chunk,
            # first in line) and GpSimd (k=1 chunk) so it lands ~4us sooner
            # than a single SWDGE transfer would. Then tiny biases (they gate
            # the first PSUM evacuations and the ACT table load), w1, and w2
            # split across both queues so everything is resident in time.
            a_r = a_inT.rearrange("(k p) b -> p k b", p=128)
            a0_sb = io.tile([128, KIN, NT], bf16, tag="a_sb")
            nc.sync.dma_start(out=a0_sb[:, 0, :], in_=a_r[:, 0, 0:NT])
            # w1 head (m=0..3 chunks) right behind tile-0's k=0 input — these
            # two transfers gate the first matmul; biases are only needed by
            # the first evacuation ~1us later so they follow
            w1_sb = wpool.tile([128, KIN, H], bf16)
            w1_r = w1.rearrange("(k p) h -> p k h", p=128)
            nc.sync.dma_start(out=w1_sb[:, :, 0:512], in_=w1_r[:, :, 0:512])
            b12_sb = wpool.tile([128, 2 * MH], f32)
            nc.sync.dma_start(out=b12_sb, in_=b12[:, :])
            bst_sb = wpool.tile([SPLIT, 2], f32)
            nc.sync.dma_start(out=bst_sb, in_=bst[:, :])
            b1_sb = b12_sb[:, 0:MH]
            b2_sb = b12_sb[:, MH:2 * MH]
            nc.sync.dma_start(out=w1_sb[:, :, 512:], in_=w1_r[:, :, 512:])
            w2_sb = wpool.tile([128, KH, H], bf16)
            w2_r = w2.rearrange("(k p) h -> p k h", p=128)
            nc.sync.dma_start(out=w2_sb[:, :KH // 2, :],
                              in_=w2_r[:, :KH // 2, :])
            ws_sb = wpool.tile([128, KH, SPLIT], bf16)
            nc.sync.dma_start(
                out=ws_sb, in_=ws.rearrange("(k p) s -> p k s", p=128))
            wt_sb = wpool.tile([128, KH, SPLIT], bf16)
            nc.sync.dma_start(
                out=wt_sb, in_=wt.rearrange("(k p) s -> p k s", p=128))
            # GpSimd queue: tile-0's k=1 input chunk, then w2's second half
            # (needed by layer 2 of tile 0 at ~21us)
            nc.gpsimd.dma_start(out=a0_sb[:, 1, :], in_=a_r[:, 1, 0:NT])
            nc.gpsimd.dma_start(out=w2_sb[:, KH // 2:, :],
                                in_=w2_r[:, KH // 2:, :])

            ones_sb = wpool.tile([128, 1], bf16)
            nc.vector.memset(ones_sb, 1.0)
            zeros_sb = wpool.tile([128, NT], bf16)
            nc.vector.memset(zeros_sb, 0.0)

            # warm-up matmuls on scratch data: run during the initial DMA
            # wait so the PE's HAM clock gate and instruction fetch are warm
            # when the first real matmul issues
            ps_warm = psum.tile([1, NT], f32, bufs=1, tag="ps_ld")
            for _ in range(10):
                nc.tensor.matmul(ps_warm, ones_sb, zeros_sb,
                                 start=True, stop=True)

            def relu_evac(dst, ps, bias, m):
                # alternate engines so consecutive chunks drain in parallel
                if m % 2 == 0:
                    nc.scalar.activation(dst, ps, ACT.Relu, bias=bias)
                else:
                    nc.vector.scalar_tensor_tensor(
                        dst, ps, bias, zeros_sb, op0=ALU.add, op1=ALU.max)

            def emit_tail(pending, last=False):
                # deferred elementwise tail of the previous tile: runs while
                # the next tile's layer 2 keeps the PE busy. mul/add are
                # pure-SBUF so they ride the idle GpSimd engine, keeping
                # VectorE free for h1 evacuations at the L2 phase boundary.
                log_s_p, t_p, x2u_p, n0_p = pending
                es = misc.tile([SPLIT, NT], f32)
                nc.scalar.activation(es, log_s_p, ACT.Exp)
                prod = misc.tile([SPLIT, NT], f32)
                x2t_sb = misc.tile([SPLIT, NT], f32)
                if last:
                    # final flush is the kernel tail: use VectorE and split in
                    # quarters so compute pipelines with the stores
                    q = NT // 4
                    for h0, h1 in [(i * q, (i + 1) * q) for i in range(4)]:
                        nc.vector.tensor_mul(prod[:, h0:h1], x2u_p[:, h0:h1],
                                             es[:, h0:h1])
                        nc.vector.tensor_add(x2t_sb[:, h0:h1],
                                             prod[:, h0:h1], t_p[:, h0:h1])
                        nc.sync.dma_start(out=x2tT[:, n0_p + h0:n0_p + h1],
                                          in_=x2t_sb[:, h0:h1])
                else:
                    nc.gpsimd.tensor_mul(prod, x2u_p, es)
                    nc.gpsimd.tensor_add(x2t_sb, prod, t_p)
                    nc.sync.dma_start(out=x2tT[:, n0_p:n0_p + NT], in_=x2t_sb)

            pending = None
            for it in range(NTILES):
                n0 = it * NT

                if it == 0:
                    a_sb = a0_sb
                    # tile-0's x2u isn't needed until its deferred tail
                    # (~35us); it rides Sync behind the weights
                    x2u_sb = io.tile([SPLIT, NT], f32)
                    nc.sync.dma_start(out=x2u_sb, in_=x2uT[:, 0:NT])
                else:
                    a_sb = io.tile([128, KIN, NT], bf16)
                    nc.gpsimd.dma_start(out=a_sb, in_=a_r[:, :, n0:n0 + NT])
                    x2u_sb = io.tile([SPLIT, NT], f32)
                    nc.gpsimd.dma_start(out=x2u_sb, in_=x2uT[:, n0:n0 + NT])

                # ---- layer 1: h1 = relu(W1^T a + b1), feature-major ----
                h1_sb = hpool.tile([128, KH, NT], bf16)
                for m in range(MH):
                    ps = psum.tile([128, NT], f32)
                    for k in range(KIN):
                        nc.tensor.matmul(
                            ps, w1_sb[:, k, m * 128:(m + 1) * 128],
                            a_sb[:, k, :],
                            start=(k == 0), stop=(k == KIN - 1))
                    relu_evac(h1_sb[:, m, :], ps, b1_sb[:, m:m + 1], m)

                if pending is not None:
                    emit_tail(pending)
                    pending = None

                # ---- layer 2: h2 = relu(W2^T h1 + b2) ----
                h2_sb = hpool.tile([128, KH, NT], bf16)
                for m in range(MH):
                    ps = psum.tile([128, NT], f32)
                    for k in range(KH):
                        nc.tensor.matmul(
                            ps, w2_sb[:, k, m * 128:(m + 1) * 128],
                            h1_sb[:, k, :],
                            start=(k == 0), stop=(k == KH - 1))
                    relu_evac(h2_sb[:, m, :], ps, b2_sb[:, m:m + 1], m + 1)

                # ---- heads: log_s = tanh(Ws^T h2 + bs); t = Wt^T h2 + bt ----
                ps_s = psum.tile([SPLIT, NT], f32, bufs=2)
                for k in range(KH):
                    nc.tensor.matmul(ps_s, ws_sb[:, k, :], h2_sb[:, k, :],
                                     start=(k == 0), stop=(k == KH - 1))
                log_s_sb = misc.tile([SPLIT, NT], bf16)
                nc.scalar.activation(log_s_sb, ps_s, ACT.Tanh,
                                     bias=bst_sb[:, 0:1])
                # log_s ships as soon as tanh lands — keeps it off the
                # deferred tail and out of the kernel's final drain
                nc.sync.dma_start(out=log_sT[:, n0:n0 + NT], in_=log_s_sb)

                ps_t = psum.tile([SPLIT, NT], f32, bufs=1)
                for k in range(KH):
                    nc.tensor.matmul(ps_t, wt_sb[:, k, :], h2_sb[:, k, :],
                                     start=(k == 0), stop=(k == KH - 1))
                # evacuate t promptly via ScalarE (frees the PSUM bank so the
                # next iteration's Lt can run with ps_t single-buffered)
                t_sb = misc.tile([SPLIT, NT], f32)
                nc.scalar.activation(t_sb, ps_t, ACT.Identity,
                                     bias=bst_sb[:, 1:2])

                pending = (log_s_sb, t_sb, x2u_sb, n0)

            emit_tail(pending, last=True)

    nc.compile()
    return nc


def _get_nc():
    if "nc" not in _CACHE:
        _CACHE["nc"] = _build_nc()
    return _CACHE["nc"]


def _prepare_in_maps(inputs):
    bf16 = ml_dtypes.bfloat16
    x = np.asarray(inputs["x"], dtype=np.float32)
    cond = np.asarray(inputs["cond"], dtype=np.float32)
    w1 = np.asarray(inputs["W1"], dtype=np.float32).astype(bf16)
    w2 = np.asarray(inputs["W2"], dtype=np.float32).astype(bf16)
    ws = np.asarray(inputs["Ws"], dtype=np.float32).astype(bf16)
    wt = np.asarray(inputs["Wt"], dtype=np.float32).astype(bf16)
    b1 = np.asarray(inputs["b1"], dtype=np.float32).reshape(MH, 128).T
    b2 = np.asarray(inputs["b2"], dtype=np.float32).reshape(MH, 128).T
    b12 = np.ascontiguousarray(np.concatenate([b1, b2], axis=1))
    bst = np.ascontiguousarray(np.stack(
        [np.asarray(inputs["bs"], dtype=np.float32),
         np.asarray(inputs["bt"], dtype=np.float32)], axis=1))

    shared = {"w1": w1, "w2": w2, "ws": ws, "wt": wt, "b12": b12, "bst": bst}

    in_maps = []
    for c in range(N_CORES):
        xs = x[c * BS:(c + 1) * BS]
        cs = cond[c * BS:(c + 1) * BS]
        a_inT = np.concatenate([xs[:, 0::2], cs], axis=1).T.astype(bf16)
        x2uT = np.ascontiguousarray(xs[:, 1::2].T)
        in_maps.append({"a_inT": np.ascontiguousarray(a_inT),
                        "x2uT": x2uT, **shared})
    return in_maps


def _axon_reset():
    try:
        import ctypes

        lib = ctypes.CDLL("/opt/axon/libaxon_pjrt.so")
        lib.axon_reset.restype = ctypes.c_int64
        lib.axon_reset()
    except Exception:
        pass


def _run(inputs, trace=False, **spmd_kwargs):
    from concourse.bass_utils import run_bass_kernel_spmd

    nc = _get_nc()
    in_maps = _prepare_in_maps(inputs)
    try:
        res = run_bass_kernel_spmd(nc, in_maps, core_ids=list(range(N_CORES)),
                                   trace=trace, **spmd_kwargs)
    except Exception:
        # a wedged NeuronCore ("NRT_EXEC_UNIT_UNRECOVERABLE") survives
        # process restarts but clears with an axon client reset; retry once
        _axon_reset()
        res = run_bass_kernel_spmd(nc, in_maps, core_ids=list(range(N_CORES)),
                                   trace=trace, **spmd_kwargs)

    x = np.asarray(inputs["x"], dtype=np.float32)
    out = np.empty((B, D), dtype=np.float32)
    out[:, 0::2] = x[:, 0::2]
    log_det = np.empty((B,), dtype=np.float32)
    for c in range(N_CORES):
        r = res.results[c]
        out[c * BS:(c + 1) * BS, 1::2] = r["x2tT"].T
        log_det[c * BS:(c + 1) * BS] = \
            r["log_sT"].astype(np.float32).sum(axis=0)
    return (out, log_det), res


def kernel(**inputs):
    (out, log_det), _ = _run(inputs, trace=False)
    return out, log_det


# revision 39
# speedup vs baseline: 1.0025x; 1.0025x over previous
"""Trainium2 Bass kernel for nn_AffineCouplingLayer.

Reference computation (B=65536, D=256, C=128, H=1024, SPLIT=128):
    x1u = x[:, 0::2]; x2u = x[:, 1::2]
    h = relu([x1u, cond] @ W1 + b1)
    h = relu(h @ W2 + b2)
    log_s = tanh(h @ Ws + bs);  t = h @ Wt + bt
    x2t = x2u * exp(log_s) + t
    out[:, 0::2] = x1u; out[:, 1::2] = x2t
    log_det = log_s.sum(axis=1)

Strategy: pure data parallel over 8 cores (batch dim), weights replicated.
On-chip layout is feature-major ("transposed"): activations live as
[feature(partition=128), batch(free)] tiles so every layer's output is
directly the next layer's matmul rhs with zero transposes. Matmuls run in
bf16 (full PE rate vs 1/4 for fp32) with fp32 PSUM accumulation. The host
pre-transposes the inputs / un-transposes the outputs and passes
x[:, 0::2] through untouched (only the odd columns are computed on device).
log_det (a partition-dim reduction) is a ones-vector bf16 matmul.

PSUM evacuations alternate between ScalarE (ACT, fused bias+relu) and
VectorE (DVE scalar_tensor_tensor, (psum+bias) max 0) so consecutive
chunks drain in parallel and neither engine gates the PE. Input loads run
on the GpSimd queue, weight loads on Sync, so the startup DMA wall
parallelizes; weight tensors are loaded with one coalesced DMA each.
"""

import numpy as np
import ml_dtypes

N_CORES = 8
B, D, C, H = 65536, 256, 128, 1024
SPLIT = D // 2            # 128
NET_IN = (D - SPLIT) + C  # 256
BS = B // N_CORES         # 8192 rows per core
NT = 512                  # batch-tile (matmul moving free dim, 1 PSUM bank fp32)
NTILES = BS // NT         # 16
KH = H // 128             # 8 contraction chunks for H
MH = H // 128             # 8 output chunks for H
KIN = NET_IN // 128       # 2 contraction chunks for layer 1

_CACHE = {}


def _build_nc():
    import concourse.mybir as mybir
    import concourse.tile as tile
    from concourse import bacc

    f32 = mybir.dt.float32
    bf16 = mybir.dt.bfloat16
    ACT = mybir.ActivationFunctionType
    ALU = mybir.AluOpType

    nc = bacc.Bacc("TRN2", target_bir_lowering=False, debug=False,
                   num_devices=N_CORES)

    # Per-core inputs (already transposed/cast host-side)
    a_inT = nc.dram_tensor("a_inT", [NET_IN, BS], bf16, kind="ExternalInput")
    x2uT = nc.dram_tensor("x2uT", [SPLIT, BS], f32, kind="ExternalInput")
    w1 = nc.dram_tensor("w1", [NET_IN, H], bf16, kind="ExternalInput")
    w2 = nc.dram_tensor("w2", [H, H], bf16, kind="ExternalInput")
    ws = nc.dram_tensor("ws", [H, SPLIT], bf16, kind="ExternalInput")
    wt = nc.dram_tensor("wt", [H, SPLIT], bf16, kind="ExternalInput")
    b12 = nc.dram_tensor("b12", [128, 2 * MH], f32, kind="ExternalInput")
    bst = nc.dram_tensor("bst", [SPLIT, 2], f32, kind="ExternalInput")

    x2tT = nc.dram_tensor("x2tT", [SPLIT, BS], f32, kind="ExternalOutput")
    # log_s ships raw (bf16); the host does the 128-feature sum. This keeps
    # the partition-dim reduction off the TensorE stream entirely (16 fewer
    # full-cost matmuls).
    log_sT = nc.dram_tensor("log_sT", [SPLIT, BS], bf16, kind="ExternalOutput")

    with tile.TileContext(nc) as tc:
        with (
            tc.tile_pool(name="wpool", bufs=1) as wpool,
            tc.tile_pool(name="io", bufs=3) as io,
            tc.tile_pool(name="hpool", bufs=2) as hpool,
            tc.tile_pool(name="misc", bufs=2) as misc,
            tc.tile_pool(name="psum", bufs=4, space="PSUM") as psum,
        ):
            # ---- resident weights / biases (coalesced DMAs on Sync) ----
            # The first real matmul needs tile-0's activations + w1's head;
            # split tile-0's input across the fast Sync queue (k=0 chunk,
            # first in line) and GpSimd (k=1 chunk) so it lands ~4us sooner
            # than a single SWDGE transfer would. Then tiny biases (they gate
            # the first PSUM evacuations and the ACT table load), w1, and w2
            # split across both queues so everything is resident in time.
            a_r = a_inT.rearrange("(k p) b -> p k b", p=128)
            a0_sb = io.tile([128, KIN, NT], bf16, tag="a_sb")
            nc.sync.dma_start(out=a0_sb[:, 0, :], in_=a_r[:, 0, 0:NT])
            # w1 head (m=0..3 chunks) right behind tile-0's k=0 input — these
            # two transfers gate the first matmul; biases are only needed by
            # the first evacuation ~1us later so they follow
            w1_sb = wpool.tile([128, KIN, H], bf16)
            w1_r = w1.rearrange("(k p) h -> p k h", p=128)
            nc.sync.dma_start(out=w1_sb[:, :, 0:512], in_=w1_r[:, :, 0:512])
            b12_sb = wpool.tile([128, 2 * MH], f32)
            nc.sync.dma_start(out=b12_sb, in_=b12[:, :])
            bst_sb = wpool.tile([SPLIT, 2], f32)
            nc.sync.dma_start(out=bst_sb, in_=bst[:, :])
            b1_sb = b12_sb[:, 0:MH]
            b2_sb = b12_sb[:, MH:2 * MH]
            nc.sync.dma_start(out=w1_sb[:, :, 512:], in_=w1_r[:, :, 512:])
            w2_sb = wpool.tile([128, KH, H], bf16)
            w2_r = w2.rearrange("(k p) h -> p k h", p=128)
            nc.sync.dma_start(out=w2_sb[:, :KH // 2, :],
                              in_=w2_r[:, :KH // 2, :])
            ws_sb = wpool.tile([128, KH, SPLIT], bf16)
            nc.sync.dma_start(
                out=ws_sb, in_=ws.rearrange("(k p) s -> p k s", p=128))
            wt_sb = wpool.tile([128, KH, SPLIT], bf16)
            nc.sync.dma_start(
                out=wt_sb, in_=wt.rearrange("(k p) s -> p k s", p=128))
            # GpSimd queue: tile-0's k=1 input chunk, then w2's second half
            # (needed by layer 2 of tile 0 at ~21us)
            nc.gpsimd.dma_start(out=a0_sb[:, 1, :], in_=a_r[:, 1, 0:NT])
            nc.gpsimd.dma_start(out=w2_sb[:, KH // 2:, :],
                                in_=w2_r[:, KH // 2:, :])

            ones_sb = wpool.tile([128, 1], bf16)
            nc.vector.memset(ones_sb, 1.0)
            zeros_sb = wpool.tile([128, NT], bf16)
            nc.vector.memset(zeros_sb, 0.0)

            # warm-up matmuls on scratch data: run during the initial DMA
            # wait so the PE's HAM clock gate and instruction fetch are warm
            # when the first real matmul issues
            ps_warm = psum.tile([1, NT], f32, bufs=1, tag="ps_ld")
            for _ in range(10):
                nc.tensor.matmul(ps_warm, ones_sb, zeros_sb,
                                 start=True, stop=True)

            def relu_evac(dst, ps, bias, m):
                # alternate engines so consecutive chunks drain in parallel
                if m % 2 == 0:
                    nc.scalar.activation(dst, ps, ACT.Relu, bias=bias)
                else:
                    nc.vector.scalar_tensor_tensor(
                        dst, ps, bias, zeros_sb, op0=ALU.add, op1=ALU.max)

            def emit_tail(pending, last=False):
                # deferred elementwise tail of the previous tile: runs while
                # the next tile's layer 2 keeps the PE busy. mul/add are
                # pure-SBUF so they ride the idle GpSimd engine, keeping
                # VectorE free for h1 evacuations at the L2 phase boundary.
                log_s_p, t_p, x2u_p, n0_p = pending
                es = misc.tile([SPLIT, NT], f32)
                nc.scalar.activation(es, log_s_p, ACT.Exp)
                prod = misc.tile([SPLIT, NT], f32)
                x2t_sb = misc.tile([SPLIT, NT], f32)
                if last:
                    # final flush is the kernel tail: use VectorE and split in
                    # quarters so compute pipelines with the stores
                    q = NT // 4
                    for h0, h1 in [(i * q, (i + 1) * q) for i in range(4)]:
                        nc.vector.tensor_mul(prod[:, h0:h1], x2u_p[:, h0:h1],
                                             es[:, h0:h1])
                        nc.vector.tensor_add(x2t_sb[:, h0:h1],
                                             prod[:, h0:h1], t_p[:, h0:h1])
                        nc.sync.dma_start(out=x2tT[:, n0_p + h0:n0_p + h1],
                                          in_=x2t_sb[:, h0:h1])
                else:
                    nc.gpsimd.tensor_mul(prod, x2u_p, es)
                    nc.gpsimd.tensor_add(x2t_sb, prod, t_p)
                    nc.sync.dma_start(out=x2tT[:, n0_p:n0_p + NT], in_=x2t_sb)

            pending = None
            for it in range(NTILES):
                n0 = it * NT

                if it == 0:
                    a_sb = a0_sb
                    # tile-0's x2u isn't needed until its deferred tail
                    # (~35us); it rides Sync behind the weights
                    x2u_sb = io.tile([SPLIT, NT], f32)
                    nc.sync.dma_start(out=x2u_sb, in_=x2uT[:, 0:NT])
                else:
                    a_sb = io.tile([128, KIN, NT], bf16)
                    nc.gpsimd.dma_start(out=a_sb, in_=a_r[:, :, n0:n0 + NT])
                    x2u_sb = io.tile([SPLIT, NT], f32)
                    nc.gpsimd.dma_start(out=x2u_sb, in_=x2uT[:, n0:n0 + NT])

                # ---- layer 1: h1 = relu(W1^T a + b1), feature-major ----
                h1_sb = hpool.tile([128, KH, NT], bf16)
                for m in range(MH):
                    ps = psum.tile([128, NT], f32)
                    for k in range(KIN):
                        nc.tensor.matmul(
                            ps, w1_sb[:, k, m * 128:(m + 1) * 128],
                            a_sb[:, k, :],
                            start=(k == 0), stop=(k == KIN - 1))
                    relu_evac(h1_sb[:, m, :], ps, b1_sb[:, m:m + 1], m)

                # ---- layer 2: h2 = relu(W2^T h1 + b2) ----
                h2_sb = hpool.tile([128, KH, NT], bf16)
                for m in range(MH):
                    ps = psum.tile([128, NT], f32)
                    for k in range(KH):
                        nc.tensor.matmul(
                            ps, w2_sb[:, k, m * 128:(m + 1) * 128],
                            h1_sb[:, k, :],
                            start=(k == 0), stop=(k == KH - 1))
                    relu_evac(h2_sb[:, m, :], ps, b2_sb[:, m:m + 1], m + 1)
                    if m == 1 and pending is not None:
                        # previous tile's elementwise tail: emitted two
                        # m-groups into L2 so its ScalarE exp queues behind
                        # (not ahead of) the h1 evacuations L2's start needs
                        emit_tail(pending)
                        pending = None

                # ---- heads: log_s = tanh(Ws^T h2 + bs); t = Wt^T h2 + bt ----
                ps_s = psum.tile([SPLIT, NT], f32, bufs=2)
                for k in range(KH):
                    nc.tensor.matmul(ps_s, ws_sb[:, k, :], h2_sb[:, k, :],
                                     start=(k == 0), stop=(k == KH - 1))
                log_s_sb = misc.tile([SPLIT, NT], bf16)
                nc.scalar.activation(log_s_sb, ps_s, ACT.Tanh,
                                     bias=bst_sb[:, 0:1])
                # log_s ships as soon as tanh lands — keeps it off the
                # deferred tail and out of the kernel's final drain
                nc.sync.dma_start(out=log_sT[:, n0:n0 + NT], in_=log_s_sb)

                ps_t = psum.tile([SPLIT, NT], f32, bufs=1)
                for k in range(KH):
                    nc.tensor.matmul(ps_t, wt_sb[:, k, :], h2_sb[:, k, :],
                                     start=(k == 0), stop=(k == KH - 1))
                # evacuate t promptly via ScalarE (frees the PSUM bank so the
                # next iteration's Lt can run with ps_t single-buffered)
                t_sb = misc.tile([SPLIT, NT], f32)
                nc.scalar.activation(t_sb, ps_t, ACT.Identity,
                                     bias=bst_sb[:, 1:2])

                pending = (log_s_sb, t_sb, x2u_sb, n0)

            emit_tail(pending, last=True)

    nc.compile()
    return nc


def _get_nc():
    if "nc" not in _CACHE:
        _CACHE["nc"] = _build_nc()
    return _CACHE["nc"]


def _prepare_in_maps(inputs):
    bf16 = ml_dtypes.bfloat16
    x = np.asarray(inputs["x"], dtype=np.float32)
    cond = np.asarray(inputs["cond"], dtype=np.float32)
    w1 = np.asarray(inputs["W1"], dtype=np.float32).astype(bf16)
    w2 = np.asarray(inputs["W2"], dtype=np.float32).astype(bf16)
    ws = np.asarray(inputs["Ws"], dtype=np.float32).astype(bf16)
    wt = np.asarray(inputs["Wt"], dtype=np.float32).astype(bf16)
    b1 = np.asarray(inputs["b1"], dtype=np.float32).reshape(MH, 128).T
    b2 = np.asarray(inputs["b2"], dtype=np.float32).reshape(MH, 128).T
    b12 = np.ascontiguousarray(np.concatenate([b1, b2], axis=1))
    bst = np.ascontiguousarray(np.stack(
        [np.asarray(inputs["bs"], dtype=np.float32),
         np.asarray(inputs["bt"], dtype=np.float32)], axis=1))

    shared = {"w1": w1, "w2": w2, "ws": ws, "wt": wt, "b12": b12, "bst": bst}

    in_maps = []
    for c in range(N_CORES):
        xs = x[c * BS:(c + 1) * BS]
        cs = cond[c * BS:(c + 1) * BS]
        a_inT = np.concatenate([xs[:, 0::2], cs], axis=1).T.astype(bf16)
        x2uT = np.ascontiguousarray(xs[:, 1::2].T)
        in_maps.append({"a_inT": np.ascontiguousarray(a_inT),
                        "x2uT": x2uT, **shared})
    return in_maps


def _axon_reset():
    try:
        import ctypes

        lib = ctypes.CDLL("/opt/axon/libaxon_pjrt.so")
        lib.axon_reset.restype = ctypes.c_int64
        lib.axon_reset()
    except Exception:
        pass


def _run(inputs, trace=False, **spmd_kwargs):
    from concourse.bass_utils import run_bass_kernel_spmd

    nc = _get_nc()
    in_maps = _prepare_in_maps(inputs)
    try:
        res = run_bass_kernel_spmd(nc, in_maps, core_ids=list(range(N_CORES)),
                                   trace=trace, **spmd_kwargs)
    except Exception:
        # a wedged NeuronCore ("NRT_EXEC_UNIT_UNRECOVERABLE") survives
        # process restarts but clears with an axon client reset; retry once
        _axon_reset()
        res = run_bass_kernel_spmd(nc, in_maps, core_ids=list(range(N_CORES)),
                                   trace=trace, **spmd_kwargs)

    x = np.asarray(inputs["x"], dtype=np.float32)
    out = np.empty((B, D), dtype=np.float32)
    out[:, 0::2] = x[:, 0::2]
    log_det = np.empty((B,), dtype=np.float32)
    for c in range(N_CORES):
        r = res.results[c]
        out[c * BS:(c + 1) * BS, 1::2] = r["x2tT"].T
        log_det[c * BS:(c + 1) * BS] = \
            r["log_sT"].astype(np.float32).sum(axis=0)
    return (out, log_det), res


def kernel(**inputs):
    (out, log_det), _ = _run(inputs, trace=False)
    return out, log_det


# revision 40
# speedup vs baseline: 1.0066x; 1.0041x over previous
"""Trainium2 Bass kernel for nn_AffineCouplingLayer.

Reference computation (B=65536, D=256, C=128, H=1024, SPLIT=128):
    x1u = x[:, 0::2]; x2u = x[:, 1::2]
    h = relu([x1u, cond] @ W1 + b1)
    h = relu(h @ W2 + b2)
    log_s = tanh(h @ Ws + bs);  t = h @ Wt + bt
    x2t = x2u * exp(log_s) + t
    out[:, 0::2] = x1u; out[:, 1::2] = x2t
    log_det = log_s.sum(axis=1)

Strategy: pure data parallel over 8 cores (batch dim), weights replicated.
On-chip layout is feature-major ("transposed"): activations live as
[feature(partition=128), batch(free)] tiles so every layer's output is
directly the next layer's matmul rhs with zero transposes. Matmuls run in
bf16 (full PE rate vs 1/4 for fp32) with fp32 PSUM accumulation. The host
pre-transposes the inputs / un-transposes the outputs and passes
x[:, 0::2] through untouched (only the odd columns are computed on device).
log_det (a partition-dim reduction) is a ones-vector bf16 matmul.

PSUM evacuations alternate between ScalarE (ACT, fused bias+relu) and
VectorE (DVE scalar_tensor_tensor, (psum+bias) max 0) so consecutive
chunks drain in parallel and neither engine gates the PE. Input loads run
on the GpSimd queue, weight loads on Sync, so the startup DMA wall
parallelizes; weight tensors are loaded with one coalesced DMA each.
"""

import numpy as np
import ml_dtypes

N_CORES = 8
B, D, C, H = 65536, 256, 128, 1024
SPLIT = D // 2            # 128
NET_IN = (D - SPLIT) + C  # 256
BS = B // N_CORES         # 8192 rows per core
NT = 512                  # batch-tile (matmul moving free dim, 1 PSUM bank fp32)
NTILES = BS // NT         # 16
KH = H // 128             # 8 contraction chunks for H
MH = H // 128             # 8 output chunks for H
KIN = NET_IN // 128       # 2 contraction chunks for layer 1

_CACHE = {}


def _build_nc():
    import concourse.mybir as mybir
    import concourse.tile as tile
    from concourse import bacc

    f32 = mybir.dt.float32
    bf16 = mybir.dt.bfloat16
    ACT = mybir.ActivationFunctionType
    ALU = mybir.AluOpType

    nc = bacc.Bacc("TRN2", target_bir_lowering=False, debug=False,
                   num_devices=N_CORES)

    # Per-core inputs (already transposed/cast host-side)
    a_inT = nc.dram_tensor("a_inT", [NET_IN, BS], bf16, kind="ExternalInput")
    x2uT = nc.dram_tensor("x2uT", [SPLIT, BS], f32, kind="ExternalInput")
    w1 = nc.dram_tensor("w1", [NET_IN, H], bf16, kind="ExternalInput")
    w2 = nc.dram_tensor("w2", [H, H], bf16, kind="ExternalInput")
    ws = nc.dram_tensor("ws", [H, SPLIT], bf16, kind="ExternalInput")
    wt = nc.dram_tensor("wt", [H, SPLIT], bf16, kind="ExternalInput")
    b12 = nc.dram_tensor("b12", [128, 2 * MH], f32, kind="ExternalInput")
    bst = nc.dram_tensor("bst", [SPLIT, 2], f32, kind="ExternalInput")

    x2tT = nc.dram_tensor("x2tT", [SPLIT, BS], f32, kind="ExternalOutput")
    # log_s ships raw (bf16); the host does the 128-feature sum. This keeps
    # the partition-dim reduction off the TensorE stream entirely (16 fewer
    # full-cost matmuls).
    log_sT = nc.dram_tensor("log_sT", [SPLIT, BS], bf16, kind="ExternalOutput")

    with tile.TileContext(nc) as tc:
        with (
            tc.tile_pool(name="wpool", bufs=1) as wpool,
            tc.tile_pool(name="io", bufs=3) as io,
            tc.tile_pool(name="hpool", bufs=2) as hpool,
            tc.tile_pool(name="misc", bufs=2) as misc,
            tc.tile_pool(name="psum", bufs=4, space="PSUM") as psum,
        ):
            # ---- resident weights / biases (coalesced DMAs on Sync) ----
            # The first real matmul needs tile-0's activations + w1's head;
            # split tile-0's input across the fast Sync queue (k=0 chunk,
            # first in line) and GpSimd (k=1 chunk) so it lands ~4us sooner
            # than a single SWDGE transfer would. Then tiny biases (they gate
            # the first PSUM evacuations and the ACT table load), w1, and w2
            # split across both queues so everything is resident in time.
            a_r = a_inT.rearrange("(k p) b -> p k b", p=128)
            a0_sb = io.tile([128, KIN, NT], bf16, tag="a_sb")
            nc.sync.dma_start(out=a0_sb[:, 0, :], in_=a_r[:, 0, 0:NT])
            # w1 head (m=0..3 chunks) right behind tile-0's k=0 input — these
            # two transfers gate the first matmul; biases are only needed by
            # the first evacuation ~1us later so they follow
            w1_sb = wpool.tile([128, KIN, H], bf16)
            w1_r = w1.rearrange("(k p) h -> p k h", p=128)
            nc.sync.dma_start(out=w1_sb[:, :, 0:512], in_=w1_r[:, :, 0:512])
            b12_sb = wpool.tile([128, 2 * MH], f32)
            nc.sync.dma_start(out=b12_sb, in_=b12[:, :])
            bst_sb = wpool.tile([SPLIT, 2], f32)
            nc.sync.dma_start(out=bst_sb, in_=bst[:, :])
            b1_sb = b12_sb[:, 0:MH]
            b2_sb = b12_sb[:, MH:2 * MH]
            nc.sync.dma_start(out=w1_sb[:, :, 512:], in_=w1_r[:, :, 512:])
            w2_sb = wpool.tile([128, KH, H], bf16)
            w2_r = w2.rearrange("(k p) h -> p k h", p=128)
            nc.sync.dma_start(out=w2_sb[:, :KH // 2, :],
                              in_=w2_r[:, :KH // 2, :])
            ws_sb = wpool.tile([128, KH, SPLIT], bf16)
            nc.sync.dma_start(
                out=ws_sb, in_=ws.rearrange("(k p) s -> p k s", p=128))
            wt_sb = wpool.tile([128, KH, SPLIT], bf16)
            nc.sync.dma_start(
                out=wt_sb, in_=wt.rearrange("(k p) s -> p k s", p=128))
            # GpSimd queue: tile-0's k=1 input chunk, then w2's second half
            # (needed by layer 2 of tile 0 at ~21us)
            nc.gpsimd.dma_start(out=a0_sb[:, 1, :], in_=a_r[:, 1, 0:NT])
            nc.gpsimd.dma_start(out=w2_sb[:, KH // 2:, :],
                                in_=w2_r[:, KH // 2:, :])

            ones_sb = wpool.tile([128, 1], bf16)
            nc.vector.memset(ones_sb, 1.0)
            zeros_sb = wpool.tile([128, NT], bf16)
            nc.vector.memset(zeros_sb, 0.0)

            # warm-up matmuls on scratch data: run during the initial DMA
            # wait so the PE's HAM clock gate and instruction fetch are warm
            # when the first real matmul issues
            ps_warm = psum.tile([1, NT], f32, bufs=1, tag="ps_ld")
            for _ in range(10):
                nc.tensor.matmul(ps_warm, ones_sb, zeros_sb,
                                 start=True, stop=True)

            def relu_evac(dst, ps, bias, m):
                # alternate engines so consecutive chunks drain in parallel
                if m % 2 == 0:
                    nc.scalar.activation(dst, ps, ACT.Relu, bias=bias)
                else:
                    nc.vector.scalar_tensor_tensor(
                        dst, ps, bias, zeros_sb, op0=ALU.add, op1=ALU.max)

            def emit_tail(pending, last=False):
                # deferred elementwise tail of the previous tile: runs while
                # the next tile's layer 2 keeps the PE busy. mul/add are
                # pure-SBUF so they ride the idle GpSimd engine, keeping
                # VectorE free for h1 evacuations at the L2 phase boundary.
                log_s_p, t_p, x2u_p, n0_p = pending
                es = misc.tile([SPLIT, NT], f32)
                nc.scalar.activation(es, log_s_p, ACT.Exp)
                prod = misc.tile([SPLIT, NT], f32)
                x2t_sb = misc.tile([SPLIT, NT], f32)
                if last:
                    # final flush is the kernel tail: use VectorE and split in
                    # quarters so compute pipelines with the stores
                    q = NT // 4
                    for h0, h1 in [(i * q, (i + 1) * q) for i in range(4)]:
                        nc.vector.tensor_mul(prod[:, h0:h1], x2u_p[:, h0:h1],
                                             es[:, h0:h1])
                        nc.vector.tensor_add(x2t_sb[:, h0:h1],
                                             prod[:, h0:h1], t_p[:, h0:h1])
                        nc.sync.dma_start(out=x2tT[:, n0_p + h0:n0_p + h1],
                                          in_=x2t_sb[:, h0:h1])
                else:
                    nc.gpsimd.tensor_mul(prod, x2u_p, es)
                    nc.gpsimd.tensor_add(x2t_sb, prod, t_p)
                    nc.sync.dma_start(out=x2tT[:, n0_p:n0_p + NT], in_=x2t_sb)

            pending = None
            for it in range(NTILES):
                n0 = it * NT

                if it == 0:
                    a_sb = a0_sb
                    # tile-0's x2u isn't needed until its deferred tail
                    # (~35us); it rides Sync behind the weights
                    x2u_sb = io.tile([SPLIT, NT], f32)
                    nc.sync.dma_start(out=x2u_sb, in_=x2uT[:, 0:NT])
                else:
                    a_sb = io.tile([128, KIN, NT], bf16)
                    nc.gpsimd.dma_start(out=a_sb, in_=a_r[:, :, n0:n0 + NT])
                    x2u_sb = io.tile([SPLIT, NT], f32)
                    nc.gpsimd.dma_start(out=x2u_sb, in_=x2uT[:, n0:n0 + NT])

                # ---- layer 1: h1 = relu(W1^T a + b1), feature-major ----
                h1_sb = hpool.tile([128, KH, NT], bf16)
                for m in range(MH):
                    ps = psum.tile([128, NT], f32)
                    for k in range(KIN):
                        nc.tensor.matmul(
                            ps, w1_sb[:, k, m * 128:(m + 1) * 128],
                            a_sb[:, k, :],
                            start=(k == 0), stop=(k == KIN - 1))
                    relu_evac(h1_sb[:, m, :], ps, b1_sb[:, m:m + 1], m)

                # ---- layer 2: h2 = relu(W2^T h1 + b2) ----
                h2_sb = hpool.tile([128, KH, NT], bf16)
                for m in range(MH):
                    ps = psum.tile([128, NT], f32)
                    for k in range(KH):
                        nc.tensor.matmul(
                            ps, w2_sb[:, k, m * 128:(m + 1) * 128],
                            h1_sb[:, k, :],
                            start=(k == 0), stop=(k == KH - 1))
                    relu_evac(h2_sb[:, m, :], ps, b2_sb[:, m:m + 1], m + 1)
                    if m == 1 and pending is not None:
                        # previous tile's elementwise tail: emitted two
                        # m-groups into L2 so its ScalarE exp queues behind
                        # (not ahead of) the h1 evacuations L2's start needs
                        emit_tail(pending)
                        pending = None

                # ---- heads: log_s = tanh(Ws^T h2 + bs); t = Wt^T h2 + bt ----
                ps_s = psum.tile([SPLIT, NT], f32, bufs=2)
                for k in range(KH):
                    nc.tensor.matmul(ps_s, ws_sb[:, k, :], h2_sb[:, k, :],
                                     start=(k == 0), stop=(k == KH - 1))
                log_s_sb = misc.tile([SPLIT, NT], bf16)
                nc.scalar.activation(log_s_sb, ps_s, ACT.Tanh,
                                     bias=bst_sb[:, 0:1])
                # log_s ships as soon as tanh lands — keeps it off the
                # deferred tail and out of the kernel's final drain
                nc.sync.dma_start(out=log_sT[:, n0:n0 + NT], in_=log_s_sb)

                if it < NTILES - 1:
                    ps_t = psum.tile([SPLIT, NT], f32, bufs=1)
                    for k in range(KH):
                        nc.tensor.matmul(ps_t, wt_sb[:, k, :], h2_sb[:, k, :],
                                         start=(k == 0), stop=(k == KH - 1))
                    # evacuate t promptly via ScalarE (frees the PSUM bank so
                    # the next iteration's Lt runs with ps_t single-buffered)
                    t_sb = misc.tile([SPLIT, NT], f32)
                    nc.scalar.activation(t_sb, ps_t, ACT.Identity,
                                         bias=bst_sb[:, 1:2])
                else:
                    # final tile: Lt in two N=256 groups so the first half's
                    # t-copy/add/store pipeline under the second half's
                    # matmuls, halving the kernel's post-matmul tail. The
                    # second group borrows the warm-up's retired PSUM slot.
                    HF = NT // 2
                    t_sb = misc.tile([SPLIT, NT], f32)
                    for hi, tag in ((0, "ps_t"), (1, "ps_ld")):
                        h0 = hi * HF
                        ps_th = psum.tile([SPLIT, HF], f32, bufs=1, tag=tag)
                        for k in range(KH):
                            nc.tensor.matmul(
                                ps_th, wt_sb[:, k, :],
                                h2_sb[:, k, h0:h0 + HF],
                                start=(k == 0), stop=(k == KH - 1))
                        nc.scalar.activation(t_sb[:, h0:h0 + HF], ps_th,
                                             ACT.Identity,
                                             bias=bst_sb[:, 1:2])

                pending = (log_s_sb, t_sb, x2u_sb, n0)

            emit_tail(pending, last=True)

    nc.compile()
    return nc


def _get_nc():
    if "nc" not in _CACHE:
        _CACHE["nc"] = _build_nc()
    return _CACHE["nc"]


def _prepare_in_maps(inputs):
    bf16 = ml_dtypes.bfloat16
    x = np.asarray(inputs["x"], dtype=np.float32)
    cond = np.asarray(inputs["cond"], dtype=np.float32)
    w1 = np.asarray(inputs["W1"], dtype=np.float32).astype(bf16)
    w2 = np.asarray(inputs["W2"], dtype=np.float32).astype(bf16)
    ws = np.asarray(inputs["Ws"], dtype=np.float32).astype(bf16)
    wt = np.asarray(inputs["Wt"], dtype=np.float32).astype(bf16)
    b1 = np.asarray(inputs["b1"], dtype=np.float32).reshape(MH, 128).T
    b2 = np.asarray(inputs["b2"], dtype=np.float32).reshape(MH, 128).T
    b12 = np.ascontiguousarray(np.concatenate([b1, b2], axis=1))
    bst = np.ascontiguousarray(np.stack(
        [np.asarray(inputs["bs"], dtype=np.float32),
         np.asarray(inputs["bt"], dtype=np.float32)], axis=1))

    shared = {"w1": w1, "w2": w2, "ws": ws, "wt": wt, "b12": b12, "bst": bst}

    in_maps = []
    for c in range(N_CORES):
        xs = x[c * BS:(c + 1) * BS]
        cs = cond[c * BS:(c + 1) * BS]
        a_inT = np.concatenate([xs[:, 0::2], cs], axis=1).T.astype(bf16)
        x2uT = np.ascontiguousarray(xs[:, 1::2].T)
        in_maps.append({"a_inT": np.ascontiguousarray(a_inT),
                        "x2uT": x2uT, **shared})
    return in_maps


def _axon_reset():
    try:
        import ctypes

        lib = ctypes.CDLL("/opt/axon/libaxon_pjrt.so")
        lib.axon_reset.restype = ctypes.c_int64
        lib.axon_reset()
    except Exception:
        pass


def _run(inputs, trace=False, **spmd_kwargs):
    from concourse.bass_utils import run_bass_kernel_spmd

    nc = _get_nc()
    in_maps = _prepare_in_maps(inputs)
    try:
        res = run_bass_kernel_spmd(nc, in_maps, core_ids=list(range(N_CORES)),
                                   trace=trace, **spmd_kwargs)
    except Exception:
        # a wedged NeuronCore ("NRT_EXEC_UNIT_UNRECOVERABLE") survives
        # process restarts but clears with an axon client reset; retry once
        _axon_reset()
        res = run_bass_kernel_spmd(nc, in_maps, core_ids=list(range(N_CORES)),
                                   trace=trace, **spmd_kwargs)

    x = np.asarray(inputs["x"], dtype=np.float32)
    out = np.empty((B, D), dtype=np.float32)
    out[:, 0::2] = x[:, 0::2]
    log_det = np.empty((B,), dtype=np.float32)
    for c in range(N_CORES):
        r = res.results[c]
        out[c * BS:(c + 1) * BS, 1::2] = r["x2tT"].T
        log_det[c * BS:(c + 1) * BS] = \
            r["log_sT"].astype(np.float32).sum(axis=0)
    return (out, log_det), res


def kernel(**inputs):
    (out, log_det), _ = _run(inputs, trace=False)
    return out, log_det


# revision 43
# speedup vs baseline: 1.1948x; 1.1870x over previous
"""Trainium2 Bass kernel for nn_AffineCouplingLayer.

Reference computation (B=65536, D=256, C=128, H=1024, SPLIT=128):
    x1u = x[:, 0::2]; x2u = x[:, 1::2]
    h = relu([x1u, cond] @ W1 + b1)
    h = relu(h @ W2 + b2)
    log_s = tanh(h @ Ws + bs);  t = h @ Wt + bt
    x2t = x2u * exp(log_s) + t
    out[:, 0::2] = x1u; out[:, 1::2] = x2t
    log_det = log_s.sum(axis=1)

Strategy: pure data parallel over 8 cores (batch dim), weights replicated.
On-chip layout is feature-major ("transposed"): activations live as
[feature(partition=128), batch(free)] tiles so every layer's output is
directly the next layer's matmul rhs with zero transposes. Matmuls run in
bf16 (full PE rate vs 1/4 for fp32) with fp32 PSUM accumulation. The host
pre-transposes the inputs / un-transposes the outputs and passes
x[:, 0::2] through untouched (only the odd columns are computed on device).
log_det (a partition-dim reduction) is a ones-vector bf16 matmul.

PSUM evacuations alternate between ScalarE (ACT, fused bias+relu) and
VectorE (DVE scalar_tensor_tensor, (psum+bias) max 0) so consecutive
chunks drain in parallel and neither engine gates the PE. Input loads run
on the GpSimd queue, weight loads on Sync, so the startup DMA wall
parallelizes; weight tensors are loaded with one coalesced DMA each.
"""

import numpy as np
import ml_dtypes

N_CORES = 8
B, D, C, H = 65536, 256, 128, 1024
SPLIT = D // 2            # 128
NET_IN = (D - SPLIT) + C  # 256
BS = B // N_CORES         # 8192 rows per core
NT = 512                  # batch-tile (matmul moving free dim, 1 PSUM bank fp32)
NTILES = BS // NT         # 16
KH = H // 128             # 8 contraction chunks for H
MH = H // 128             # 8 output chunks for H
KIN = NET_IN // 128       # 2 contraction chunks for layer 1

_CACHE = {}


def _build_nc():
    import concourse.mybir as mybir
    import concourse.tile as tile
    from concourse import bacc

    f32 = mybir.dt.float32
    bf16 = mybir.dt.bfloat16
    ACT = mybir.ActivationFunctionType
    ALU = mybir.AluOpType

    nc = bacc.Bacc("TRN2", target_bir_lowering=False, debug=False,
                   num_devices=N_CORES)

    # Per-core inputs (already transposed/cast host-side)
    a_inT = nc.dram_tensor("a_inT", [NET_IN, BS], bf16, kind="ExternalInput")
    x2uT = nc.dram_tensor("x2uT", [SPLIT, BS], f32, kind="ExternalInput")
    w1 = nc.dram_tensor("w1", [NET_IN, H], bf16, kind="ExternalInput")
    w2 = nc.dram_tensor("w2", [H, H], bf16, kind="ExternalInput")
    ws = nc.dram_tensor("ws", [H, SPLIT], bf16, kind="ExternalInput")
    wt = nc.dram_tensor("wt", [H, SPLIT], bf16, kind="ExternalInput")
    b12 = nc.dram_tensor("b12", [128, 2 * MH], f32, kind="ExternalInput")
    bst = nc.dram_tensor("bst", [SPLIT, 2], f32, kind="ExternalInput")

    x2tT = nc.dram_tensor("x2tT", [SPLIT, BS], f32, kind="ExternalOutput")
    # log_s ships raw (bf16); the host does the 128-feature sum. This keeps
    # the partition-dim reduction off the TensorE stream entirely (16 fewer
    # full-cost matmuls).
    log_sT = nc.dram_tensor("log_sT", [SPLIT, BS], bf16, kind="ExternalOutput")

    with tile.TileContext(nc) as tc:
        with (
            tc.tile_pool(name="wpool", bufs=1) as wpool,
            tc.tile_pool(name="io", bufs=3) as io,
            tc.tile_pool(name="hpool", bufs=2) as hpool,
            tc.tile_pool(name="misc", bufs=2) as misc,
            tc.tile_pool(name="psum", bufs=4, space="PSUM") as psum,
        ):
            # ---- resident weights / biases (coalesced DMAs on Sync) ----
            # The first real matmul needs tile-0's activations + w1's head;
            # split tile-0's input across the fast Sync queue (k=0 chunk,
            # first in line) and GpSimd (k=1 chunk) so it lands ~4us sooner
            # than a single SWDGE transfer would. Then tiny biases (they gate
            # the first PSUM evacuations and the ACT table load), w1, and w2
            # split across both queues so everything is resident in time.
            a_r = a_inT.rearrange("(k p) b -> p k b", p=128)
            a0_sb = io.tile([128, KIN, NT], bf16, tag="a_sb")
            nc.sync.dma_start(out=a0_sb[:, 0, :], in_=a_r[:, 0, 0:NT])
            # w1 in quarters right behind tile-0's k=0 input: the first
            # matmul waits only on the m=0..1 piece; later pieces stream in
            # just ahead of the m-groups that need them. Biases follow the
            # first piece (needed by the first evacuation ~1us later).
            w1_sb = wpool.tile([128, KIN, H], bf16)
            w1_r = w1.rearrange("(k p) h -> p k h", p=128)
            nc.sync.dma_start(out=w1_sb[:, :, 0:256], in_=w1_r[:, :, 0:256])
            b12_sb = wpool.tile([128, 2 * MH], f32)
            nc.sync.dma_start(out=b12_sb, in_=b12[:, :])
            bst_sb = wpool.tile([SPLIT, 2], f32)
            nc.sync.dma_start(out=bst_sb, in_=bst[:, :])
            b1_sb = b12_sb[:, 0:MH]
            b2_sb = b12_sb[:, MH:2 * MH]
            for q0 in (256, 512, 768):
                nc.sync.dma_start(out=w1_sb[:, :, q0:q0 + 256],
                                  in_=w1_r[:, :, q0:q0 + 256])
            w2_sb = wpool.tile([128, KH, H], bf16)
            w2_r = w2.rearrange("(k p) h -> p k h", p=128)
            nc.sync.dma_start(out=w2_sb[:, :KH // 2, :],
                              in_=w2_r[:, :KH // 2, :])
            ws_sb = wpool.tile([128, KH, SPLIT], bf16)
            nc.sync.dma_start(
                out=ws_sb, in_=ws.rearrange("(k p) s -> p k s", p=128))
            wt_sb = wpool.tile([128, KH, SPLIT], bf16)
            nc.sync.dma_start(
                out=wt_sb, in_=wt.rearrange("(k p) s -> p k s", p=128))
            # GpSimd queue: tile-0's k=1 input chunk, then w2's second half
            # (needed by layer 2 of tile 0 at ~21us), then tile-0's x2u
            # (needed by its deferred tail at ~36us)
            nc.gpsimd.dma_start(out=a0_sb[:, 1, :], in_=a_r[:, 1, 0:NT])
            nc.gpsimd.dma_start(out=w2_sb[:, KH // 2:, :],
                                in_=w2_r[:, KH // 2:, :])
            x2u0_sb = io.tile([SPLIT, NT], f32, tag="x2u_sb")
            nc.gpsimd.dma_start(out=x2u0_sb, in_=x2uT[:, 0:NT])

            ones_sb = wpool.tile([128, 1], bf16)
            nc.vector.memset(ones_sb, 1.0)
            zeros_sb = wpool.tile([128, NT], bf16)
            nc.vector.memset(zeros_sb, 0.0)

            # warm-up matmuls on scratch data: run during the initial DMA
            # wait so the PE's HAM clock gate and instruction fetch are warm
            # when the first real matmul issues
            ps_warm = psum.tile([1, NT], f32, bufs=1, tag="ps_ld")
            for _ in range(10):
                nc.tensor.matmul(ps_warm, ones_sb, zeros_sb,
                                 start=True, stop=True)

            def relu_evac(dst, ps, bias, m):
                # alternate engines so consecutive chunks drain in parallel
                if m % 2 == 0:
                    nc.scalar.activation(dst, ps, ACT.Relu, bias=bias)
                else:
                    nc.vector.scalar_tensor_tensor(
                        dst, ps, bias, zeros_sb, op0=ALU.add, op1=ALU.max)

            def emit_tail(pending, last=False):
                # deferred elementwise tail of the previous tile: runs while
                # the next tile's layer 2 keeps the PE busy. mul/add are
                # pure-SBUF so they ride the idle GpSimd engine, keeping
                # VectorE free for h1 evacuations at the L2 phase boundary.
                log_s_p, t_p, x2u_p, n0_p = pending
                es = misc.tile([SPLIT, NT], f32)
                nc.scalar.activation(es, log_s_p, ACT.Exp)
                prod = misc.tile([SPLIT, NT], f32)
                x2t_sb = misc.tile([SPLIT, NT], f32)
                if last:
                    # final flush is the kernel tail: use VectorE and split in
                    # quarters so compute pipelines with the stores
                    q = NT // 4
                    for h0, h1 in [(i * q, (i + 1) * q) for i in range(4)]:
                        nc.vector.tensor_mul(prod[:, h0:h1], x2u_p[:, h0:h1],
                                             es[:, h0:h1])
                        nc.vector.tensor_add(x2t_sb[:, h0:h1],
                                             prod[:, h0:h1], t_p[:, h0:h1])
                        nc.sync.dma_start(out=x2tT[:, n0_p + h0:n0_p + h1],
                                          in_=x2t_sb[:, h0:h1])
                else:
                    nc.gpsimd.tensor_mul(prod, x2u_p, es)
                    nc.gpsimd.tensor_add(x2t_sb, prod, t_p)
                    nc.sync.dma_start(out=x2tT[:, n0_p:n0_p + NT], in_=x2t_sb)

            pending = None
            for it in range(NTILES):
                n0 = it * NT

                if it == 0:
                    a_sb = a0_sb
                    x2u_sb = x2u0_sb
                else:
                    a_sb = io.tile([128, KIN, NT], bf16)
                    nc.gpsimd.dma_start(out=a_sb, in_=a_r[:, :, n0:n0 + NT])
                    x2u_sb = io.tile([SPLIT, NT], f32)
                    nc.gpsimd.dma_start(out=x2u_sb, in_=x2uT[:, n0:n0 + NT])

                # ---- layer 1: h1 = relu(W1^T a + b1), feature-major ----
                h1_sb = hpool.tile([128, KH, NT], bf16)
                for m in range(MH):
                    ps = psum.tile([128, NT], f32)
                    for k in range(KIN):
                        nc.tensor.matmul(
                            ps, w1_sb[:, k, m * 128:(m + 1) * 128],
                            a_sb[:, k, :],
                            start=(k == 0), stop=(k == KIN - 1))
                    relu_evac(h1_sb[:, m, :], ps, b1_sb[:, m:m + 1], m)

                # ---- layer 2: h2 = relu(W2^T h1 + b2) ----
                h2_sb = hpool.tile([128, KH, NT], bf16)
                for m in range(MH):
                    ps = psum.tile([128, NT], f32)
                    for k in range(KH):
                        nc.tensor.matmul(
                            ps, w2_sb[:, k, m * 128:(m + 1) * 128],
                            h1_sb[:, k, :],
                            start=(k == 0), stop=(k == KH - 1))
                    relu_evac(h2_sb[:, m, :], ps, b2_sb[:, m:m + 1], m + 1)
                    if m == 1 and pending is not None:
                        # previous tile's elementwise tail: emitted two
                        # m-groups into L2 so its ScalarE exp queues behind
                        # (not ahead of) the h1 evacuations L2's start needs
                        emit_tail(pending)
                        pending = None

                # ---- heads: log_s = tanh(Ws^T h2 + bs); t = Wt^T h2 + bt ----
                ps_s = psum.tile([SPLIT, NT], f32, bufs=2)
                for k in range(KH):
                    nc.tensor.matmul(ps_s, ws_sb[:, k, :], h2_sb[:, k, :],
                                     start=(k == 0), stop=(k == KH - 1))
                log_s_sb = misc.tile([SPLIT, NT], bf16)
                nc.scalar.activation(log_s_sb, ps_s, ACT.Tanh,
                                     bias=bst_sb[:, 0:1])
                # log_s ships as soon as tanh lands — keeps it off the
                # deferred tail and out of the kernel's final drain
                nc.sync.dma_start(out=log_sT[:, n0:n0 + NT], in_=log_s_sb)

                if it < NTILES - 1:
                    ps_t = psum.tile([SPLIT, NT], f32, bufs=1)
                    for k in range(KH):
                        nc.tensor.matmul(ps_t, wt_sb[:, k, :], h2_sb[:, k, :],
                                         start=(k == 0), stop=(k == KH - 1))
                    # evacuate t promptly via ScalarE (frees the PSUM bank so
                    # the next iteration's Lt runs with ps_t single-buffered)
                    t_sb = misc.tile([SPLIT, NT], f32)
                    nc.scalar.activation(t_sb, ps_t, ACT.Identity,
                                         bias=bst_sb[:, 1:2])
                else:
                    # final tile: Lt in two N=256 groups so the first half's
                    # t-copy/add/store pipeline under the second half's
                    # matmuls, halving the kernel's post-matmul tail. The
                    # second group borrows the warm-up's retired PSUM slot.
                    HF = NT // 2
                    t_sb = misc.tile([SPLIT, NT], f32)
                    for hi, tag in ((0, "ps_t"), (1, "ps_ld")):
                        h0 = hi * HF
                        ps_th = psum.tile([SPLIT, HF], f32, bufs=1, tag=tag)
                        for k in range(KH):
                            nc.tensor.matmul(
                                ps_th, wt_sb[:, k, :],
                                h2_sb[:, k, h0:h0 + HF],
                                start=(k == 0), stop=(k == KH - 1))
                        nc.scalar.activation(t_sb[:, h0:h0 + HF], ps_th,
                                             ACT.Identity,
                                             bias=bst_sb[:, 1:2])

                pending = (log_s_sb, t_sb, x2u_sb, n0)

            emit_tail(pending, last=True)

    nc.compile()
    return nc


def _get_nc():
    if "nc" not in _CACHE:
        _CACHE["nc"] = _build_nc()
    return _CACHE["nc"]


def _prepare_in_maps(inputs):
    bf16 = ml_dtypes.bfloat16
    x = np.asarray(inputs["x"], dtype=np.float32)
    cond = np.asarray(inputs["cond"], dtype=np.float32)
    w1 = np.asarray(inputs["W1"], dtype=np.float32).astype(bf16)
    w2 = np.asarray(inputs["W2"], dtype=np.float32).astype(bf16)
    ws = np.asarray(inputs["Ws"], dtype=np.float32).astype(bf16)
    wt = np.asarray(inputs["Wt"], dtype=np.float32).astype(bf16)
    b1 = np.asarray(inputs["b1"], dtype=np.float32).reshape(MH, 128).T
    b2 = np.asarray(inputs["b2"], dtype=np.float32).reshape(MH, 128).T
    b12 = np.ascontiguousarray(np.concatenate([b1, b2], axis=1))
    bst = np.ascontiguousarray(np.stack(
        [np.asarray(inputs["bs"], dtype=np.float32),
         np.asarray(inputs["bt"], dtype=np.float32)], axis=1))

    shared = {"w1": w1, "w2": w2, "ws": ws, "wt": wt, "b12": b12, "bst": bst}

    in_maps = []
    for c in range(N_CORES):
        xs = x[c * BS:(c + 1) * BS]
        cs = cond[c * BS:(c + 1) * BS]
        a_inT = np.concatenate([xs[:, 0::2], cs], axis=1).T.astype(bf16)
        x2uT = np.ascontiguousarray(xs[:, 1::2].T)
        in_maps.append({"a_inT": np.ascontiguousarray(a_inT),
                        "x2uT": x2uT, **shared})
    return in_maps


def _axon_reset():
    try:
        import ctypes

        lib = ctypes.CDLL("/opt/axon/libaxon_pjrt.so")
        lib.axon_reset.restype = ctypes.c_int64
        lib.axon_reset()
    except Exception:
        pass


def _run(inputs, trace=False, **spmd_kwargs):
    from concourse.bass_utils import run_bass_kernel_spmd

    nc = _get_nc()
    in_maps = _prepare_in_maps(inputs)
    try:
        res = run_bass_kernel_spmd(nc, in_maps, core_ids=list(range(N_CORES)),
                                   trace=trace, **spmd_kwargs)
    except Exception:
        # a wedged NeuronCore ("NRT_EXEC_UNIT_UNRECOVERABLE") survives
        # process restarts but clears with an axon client reset; retry once
        _axon_reset()
        res = run_bass_kernel_spmd(nc, in_maps, core_ids=list(range(N_CORES)),
                                   trace=trace, **spmd_kwargs)

    x = np.asarray(inputs["x"], dtype=np.float32)
    out = np.empty((B, D), dtype=np.float32)
    out[:, 0::2] = x[:, 0::2]
    log_det = np.empty((B,), dtype=np.float32)
    for c in range(N_CORES):
        r = res.results[c]
        out[c * BS:(c + 1) * BS, 1::2] = r["x2tT"].T
        log_det[c * BS:(c + 1) * BS] = \
            r["log_sT"].astype(np.float32).sum(axis=0)
    return (out, log_det), res


def kernel(**inputs):
    (out, log_det), _ = _run(inputs, trace=False)
    return out, log_det
